# revision 1
# baseline (speedup 1.0000x reference)
"""Causal self-attention (B=2, T=2048, C=1024, NH=16) on 8 trn2 NeuronCores.

Sharding: core c handles batch b = c//4 and head group g = c%4 (4 heads,
256 features). Each core computes q/k/v for its heads, causal attention in
S^T layout (keys on partitions, queries on the free dim), and a partial
output projection  y_heads @ w_proj[head_rows, :].  The host sums the four
partial projections per batch and adds b_proj.

Kernel layout notes:
  - X^T ([C, T], C on partitions) is produced on-chip with PE transposes.
  - Q^T/K^T are computed as [feat, T] tiles (2 heads per 128-partition tile),
    V as [T, feat] (natural), which feeds every later matmul without any
    further transposes:
      S^T[k, q]   = K^T-tile.T @ Q^T     (two heads packed in the PE rows)
      P^T         = exp(S^T * 1/8)       (no max subtraction; scores ~ N(0,1))
      y^T[d, q]   = V-tile.T @ P^T       (two heads packed in the PE cols)
      sums[1, q]  = ones.T @ P^T         (packed in spare PE col strips)
      out[t, o]   = y^T-tile.T @ w_proj  (partial; host reduces over cores)
  - All matmuls run as float32r (full-rate fp32 PE mode).
"""

import os
import sys

import numpy as np

for _p in ("/opt/trn_rl_repo", "/root/.axon_site/_ro/trn_rl_repo"):
    if _p not in sys.path and os.path.isdir(_p):
        sys.path.append(_p)

import concourse.bass as bass  # noqa: E402
import concourse.tile as tile  # noqa: E402
from concourse import bacc, mybir  # noqa: E402
from concourse.bass_utils import run_bass_kernel_spmd  # noqa: E402

P = 128
B, T, C = 2, 2048, 1024
NH, HD = 16, 64
HPC = 4  # heads per core
FPC = HPC * HD  # features per core (256)
QCW = 512  # query-chunk width (max fp32 moving dim)
F32 = mybir.dt.float32
F32R = mybir.dt.float32r
BF16 = mybir.dt.bfloat16


def build_nc(t_len: int = T, debug: bool = False):
    """Build the per-core Bass program (same program on all 8 cores)."""
    nt = t_len // P  # token tiles
    ncb = C // P  # contraction blocks
    nqc = t_len // QCW  # query chunks

    nc = bacc.Bacc("TRN2", target_bir_lowering=False, debug=False)

    x_d = nc.dram_tensor("x", [t_len, C], F32, kind="ExternalInput")
    wq_d = nc.dram_tensor("wq", [C, FPC], F32R, kind="ExternalInput")
    wk_d = nc.dram_tensor("wk", [C, FPC], F32R, kind="ExternalInput")
    wv_d = nc.dram_tensor("wv", [C, FPC], F32R, kind="ExternalInput")
    bqkv_d = nc.dram_tensor("bqkv", [3, FPC], F32R, kind="ExternalInput")
    wp_d = nc.dram_tensor("wp", [FPC, C], F32R, kind="ExternalInput")
    triu_d = nc.dram_tensor("triu", [P, P], F32R, kind="ExternalInput")
    ident_d = nc.dram_tensor("ident", [P, P], F32, kind="ExternalInput")
    ones_d = nc.dram_tensor("ones", [P, QCW], F32R, kind="ExternalInput")
    out_d = nc.dram_tensor("out", [t_len, C], F32, kind="ExternalOutput")

    from contextlib import ExitStack

    with tile.TileContext(nc) as tc, ExitStack() as ctx:
            consts = ctx.enter_context(tc.tile_pool(name="consts", bufs=1))
            stage = ctx.enter_context(tc.tile_pool(name="stage", bufs=2))
            bigs = ctx.enter_context(tc.tile_pool(name="bigs", bufs=1))
            xts = ctx.enter_context(tc.tile_pool(name="xts", bufs=ncb))
            qkts = ctx.enter_context(tc.tile_pool(name="qkts", bufs=4))
            yts = ctx.enter_context(tc.tile_pool(name="yts", bufs=2))
            exps = ctx.enter_context(tc.tile_pool(name="exps", bufs=3))
            smalls = ctx.enter_context(tc.tile_pool(name="smalls", bufs=1))
            psum = ctx.enter_context(tc.tile_pool(name="psum", bufs=8, space="PSUM"))
            # ---- constants / weights into SBUF ----
            triu = consts.tile([P, P], F32R)
            ident = consts.tile([P, P], F32)
            ones = consts.tile([P, QCW], F32R)
            nc.sync.dma_start(out=triu, in_=triu_d.ap())
            nc.sync.dma_start(out=ident, in_=ident_d.ap())
            nc.sync.dma_start(out=ones, in_=ones_d.ap())

            b_sb = []
            for i in range(3):
                bt = consts.tile([1, FPC], F32R, tag=f"bias{i}")
                nc.sync.dma_start(out=bt, in_=bqkv_d.ap()[i : i + 1, :])
                b_sb.append(bt)

            wq_sb = bigs.tile([P, ncb, FPC], F32R, tag="wq")
            wk_sb = bigs.tile([P, ncb, FPC], F32R, tag="wk")
            wv_sb = bigs.tile([P, ncb, FPC], F32R, tag="wv")
            for wsb, wd in ((wq_sb, wq_d), (wk_sb, wk_d), (wv_sb, wv_d)):
                nc.sync.dma_start(
                    out=wsb, in_=wd.ap().rearrange("(cb p) f -> p cb f", p=P)
                )
            wp_sb = bigs.tile([P, 2, C], F32R, tag="wp")
            nc.sync.dma_start(
                out=wp_sb, in_=wp_d.ap().rearrange("(fb p) o -> p fb o", p=P)
            )

            # ---- phase 1: X^T via PE transposes ----
            xt = [xts.tile([P, t_len], F32R, tag="xt", name=f"xt{i}") for i in range(ncb)]
            for t in range(nt):
                xst = stage.tile([P, C], F32, tag="stage")
                nc.sync.dma_start(out=xst, in_=x_d.ap()[t * P : (t + 1) * P, :])
                for cb in range(ncb):
                    ps = psum.tile([P, P], F32, tag="ps")
                    nc.tensor.transpose(ps, xst[:, cb * P : (cb + 1) * P], ident)
                    nc.vector.tensor_copy(
                        out=xt[cb][:, t * P : (t + 1) * P], in_=ps
                    )

            # ---- phase 2: Q^T, K^T ([feat, T], 2 heads/tile), V ([T, feat]) --
            qt = [qkts.tile([P, t_len], F32R, tag="qkt", name=f"qt{i}") for i in range(2)]
            kt = [qkts.tile([P, t_len], F32R, tag="qkt", name=f"kt{i}") for i in range(2)]
            for widx, wsb, dst, scale in (
                (0, wq_sb, qt, 0.125),
                (1, wk_sb, kt, None),
            ):
                for pair in range(2):
                    fs = slice(pair * P, (pair + 1) * P)
                    for qc in range(nqc):
                        cs = slice(qc * QCW, (qc + 1) * QCW)
                        ps = psum.tile([P, QCW], F32, tag="ps")
                        for cb in range(ncb):
                            nc.tensor.matmul(
                                ps,
                                (wsb[:, cb, fs]),
                                (xt[cb][:, cs]),
                                start=(cb == 0),
                                stop=False,
                            )
                        nc.tensor.matmul(
                            ps,
                            (b_sb[widx][0:1, fs]),
                            (ones[0:1, :]),
                            start=False,
                            stop=True,
                        )
                        if scale is not None:
                            nc.vector.tensor_scalar_mul(dst[pair][:, cs], ps, scale)
                        else:
                            nc.vector.tensor_copy(out=dst[pair][:, cs], in_=ps)

            # V stored as [P, nt, pair, 130]: per pair, head-A block cols 0:65
            # = [d(64), ones], head-B block cols 65:130 = [d(64), ones].  The
            # ones column makes the PV matmul also produce the softmax
            # denominator in output row 64 (M=65).
            v_sb = bigs.tile([P, nt, 2, 130], F32R, tag="v")
            for h in (64, 129):
                nc.vector.tensor_copy(
                    out=v_sb[:, :, :, h],
                    in_=ones[:, 0 : nt * 2].rearrange("p (a b) -> p a b", b=2),
                )
            for t in range(nt):
                ps = psum.tile([P, FPC], F32, tag="ps")
                for cb in range(ncb):
                    nc.tensor.matmul(
                        ps,
                        (xt[cb][:, t * P : (t + 1) * P]),
                        (wv_sb[:, cb, :]),
                        start=(cb == 0),
                        stop=False,
                    )
                nc.tensor.matmul(
                    ps,
                    (ones[0:1, 0:P]),
                    (b_sb[2][0:1, :]),
                    start=False,
                    stop=True,
                )
                nc.vector.tensor_copy(
                    out=v_sb[:, t].rearrange("p a (h w) -> p a h w", w=65)[
                        :, :, :, 0:64
                    ],
                    in_=ps.rearrange("p (a h w) -> p a h w", a=2, w=64),
                )

            # ---- phase 3: causal attention in S^T layout ----
            yt = [yts.tile([P, t_len], F32R, tag="yt", name=f"yt{i}") for i in range(2)]
            for pair in range(2):
                for qc in range(nqc):
                    cs = slice(qc * QCW, (qc + 1) * QCW)
                    cs0 = qc * QCW
                    nki = 4 * (qc + 1)
                    yA_ps = psum.tile([P, QCW], F32, tag="ps", name="yA_ps")
                    yB_ps = psum.tile([P, QCW], F32, tag="ps", name="yB_ps")
                    for ki in range(nki):
                        m = ki - 4 * qc
                        lo = max(m, 0) * P  # first unmasked column of this k-tile
                        ks = slice(ki * P, (ki + 1) * P)
                        stA = psum.tile([P, QCW], F32, tag="ps", name="stA")
                        stB = psum.tile([P, QCW], F32, tag="ps", name="stB")
                        nc.tensor.matmul(
                            stA[:, lo:],
                            kt[pair][0:64, ks],
                            qt[pair][0:64, cs0 + lo : cs0 + QCW],
                            start=True,
                            stop=True,
                        )
                        nc.tensor.matmul(
                            stB[:, lo:],
                            kt[pair][64:P, ks],
                            qt[pair][64:P, cs0 + lo : cs0 + QCW],
                            start=True,
                            stop=True,
                            tile_position=(64, 0),
                        )
                        eA = exps.tile([P, QCW], F32R, tag="exp", name="eA")
                        eB = exps.tile([P, QCW], F32R, tag="exp", name="eB")
                        nc.scalar.activation(
                            eA[:, lo:], stA[:, lo:], mybir.ActivationFunctionType.Exp
                        )
                        nc.scalar.activation(
                            eB[:, lo:], stB[:, lo:], mybir.ActivationFunctionType.Exp
                        )
                        if m >= 0:  # diagonal 128-block: causal triangle mask
                            ds_ = slice(m * P, (m + 1) * P)
                            nc.vector.tensor_mul(eA[:, ds_], eA[:, ds_], triu)
                            nc.vector.tensor_mul(eB[:, ds_], eB[:, ds_], triu)
                        if debug and pair == 0 and qc == 0 and ki in (0, 3):
                            dbgE = smalls.tile(
                                [P, QCW], F32R, tag=f"dbgE{ki}", bufs=1,
                                name=f"dbgE{ki}",
                            )
                            nc.vector.tensor_copy(out=dbgE[:, lo:], in_=eA[:, lo:])
                            d = nc.dram_tensor(
                                f"dbg_e{ki}", [P, QCW], F32R, kind="ExternalOutput"
                            )
                            nc.sync.dma_start(out=d.ap(), in_=dbgE)
                        st, sp = ki == 0, ki == nki - 1
                        nc.tensor.matmul(
                            yA_ps[0:65, lo:],
                            v_sb[:, ki, pair, 0:65],
                            eA[:, lo:],
                            start=st,
                            stop=sp,
                        )
                        nc.tensor.matmul(
                            yB_ps[0:65, lo:],
                            v_sb[:, ki, pair, 65:130],
                            eB[:, lo:],
                            start=st,
                            stop=sp,
                        )
                    if debug and pair == 0 and qc == 0:
                        for nm, src in (("dbg_ya", yA_ps), ("dbg_yb", yB_ps)):
                            dbgY = smalls.tile(
                                [P, QCW], F32, tag=nm, bufs=1, name=nm
                            )
                            nc.vector.tensor_copy(
                                out=dbgY[0:65, :], in_=src[0:65, :]
                            )
                            d = nc.dram_tensor(
                                nm, [P, QCW], F32, kind="ExternalOutput"
                            )
                            nc.sync.dma_start(out=d.ap(), in_=dbgY)
                    # Copy unnormalized y (+ sums in row 64) to SBUF right
                    # away so the PSUM banks free up for the next iteration's
                    # matmuls (the in-order PE queue stalls on slot waits).
                    yuA = smalls.tile([65, QCW], F32, tag="yuA")
                    yuB = smalls.tile([65, QCW], F32, tag="yuB")
                    nc.vector.tensor_copy(out=yuA, in_=yA_ps[0:65, :])
                    nc.vector.tensor_copy(out=yuB, in_=yB_ps[0:65, :])
                    nc.vector.reciprocal(yuA[64:65, :], yuA[64:65, :])
                    nc.vector.reciprocal(yuB[64:65, :], yuB[64:65, :])
                    recbA = smalls.tile([64, QCW], F32, tag="recbA")
                    recbB = smalls.tile([64, QCW], F32, tag="recbB")
                    nc.gpsimd.dma_start(
                        out=recbA,
                        in_=yuA[64:65, None, :].broadcast_to([1, 64, QCW]),
                    )
                    nc.gpsimd.dma_start(
                        out=recbB,
                        in_=yuB[64:65, None, :].broadcast_to([1, 64, QCW]),
                    )
                    if debug and pair == 0 and qc == 0:
                        dbgR = smalls.tile(
                            [P, QCW], F32, tag="dbg_recb", bufs=1, name="dbgR"
                        )
                        nc.vector.tensor_copy(out=dbgR[0:64, :], in_=recbA)
                        nc.vector.tensor_copy(out=dbgR[64:P, :], in_=recbB)
                        d = nc.dram_tensor(
                            "dbg_recb", [P, QCW], F32, kind="ExternalOutput"
                        )
                        nc.sync.dma_start(out=d.ap(), in_=dbgR)
                    nc.vector.tensor_mul(
                        yt[pair][0:64, cs], yuA[0:64, :], recbA
                    )
                    nc.vector.tensor_mul(
                        yt[pair][64:P, cs], yuB[0:64, :], recbB
                    )

            if debug:
                dbg_specs = [
                    ("dbg_xt", xt[0]),
                    ("dbg_qt", qt[0]),
                    ("dbg_kt", kt[0]),
                    ("dbg_yt", yt[0]),
                    ("dbg_v", v_sb.rearrange("p a b c -> p (a b c)")),
                ]
                for nm, src in dbg_specs:
                    d = nc.dram_tensor(
                        nm, [P, src.free_size()], src.dtype, kind="ExternalOutput"
                    )
                    nc.sync.dma_start(out=d.ap(), in_=src)

            # ---- phase 4: partial output projection ----
            for t in range(nt):
                ost = stage.tile([P, C], F32, tag="stage")
                for nch in range(2):
                    ps = psum.tile([P, QCW], F32, tag="ps")
                    for fb in range(2):
                        nc.tensor.matmul(
                            ps,
                            (yt[fb][:, t * P : (t + 1) * P]),
                            (wp_sb[:, fb, nch * QCW : (nch + 1) * QCW]),
                            start=(fb == 0),
                            stop=(fb == 1),
                        )
                    nc.vector.tensor_copy(
                        out=ost[:, nch * QCW : (nch + 1) * QCW], in_=ps
                    )
                nc.sync.dma_start(out=out_d.ap()[t * P : (t + 1) * P, :], in_=ost)

    nc.compile()
    return nc


_NC_CACHE: dict = {}
LAST_RESULT = None


def kernel(x, w_attn, b_attn, w_proj, b_proj):
    global LAST_RESULT
    x = np.ascontiguousarray(np.asarray(x, np.float32))
    w_attn = np.ascontiguousarray(np.asarray(w_attn, np.float32))
    b_attn = np.ascontiguousarray(np.asarray(b_attn, np.float32))
    w_proj = np.ascontiguousarray(np.asarray(w_proj, np.float32))
    b_proj = np.ascontiguousarray(np.asarray(b_proj, np.float32))

    if "nc" not in _NC_CACHE:
        _NC_CACHE["nc"] = build_nc(T)
    nc = _NC_CACHE["nc"]

    triu = np.triu(np.ones((P, P), np.float32))
    ident = np.eye(P, dtype=np.float32)
    ones = np.ones((P, QCW), np.float32)

    in_maps = []
    for core in range(8):
        b, g = core // 4, core % 4
        f0 = g * FPC
        in_maps.append(
            {
                "x": np.ascontiguousarray(x[b]),
                "wq": np.ascontiguousarray(w_attn[:, f0 : f0 + FPC]),
                "wk": np.ascontiguousarray(w_attn[:, C + f0 : C + f0 + FPC]),
                "wv": np.ascontiguousarray(
                    w_attn[:, 2 * C + f0 : 2 * C + f0 + FPC]
                ),
                "bqkv": np.stack(
                    [
                        b_attn[f0 : f0 + FPC],
                        b_attn[C + f0 : C + f0 + FPC],
                        b_attn[2 * C + f0 : 2 * C + f0 + FPC],
                    ]
                ),
                "wp": np.ascontiguousarray(w_proj[f0 : f0 + FPC, :]),
                "triu": triu,
                "ident": ident,
                "ones": ones,
            }
        )

    trace = bool(os.environ.get("BASS_TRACE"))
    res = run_bass_kernel_spmd(
        nc,
        in_maps,
        core_ids=list(range(8)),
        trace=trace,
        tmpdir=os.environ.get("KERNEL_TRACE_DIR") or None,
    )
    LAST_RESULT = res

    y = np.empty((B, T, C), np.float32)
    for b in range(B):
        acc = res.results[4 * b]["out"].astype(np.float32).copy()
        for g in range(1, 4):
            acc += res.results[4 * b + g]["out"]
        y[b] = acc + b_proj[None, :]
    return y



# revision 9
# speedup vs baseline: 1.5816x; 1.5816x over previous
"""Causal self-attention (B=2, T=2048, C=1024, NH=16) on 8 trn2 NeuronCores.

Sharding: core c handles batch b = c//4 and head group g = c%4 (4 heads,
256 features).  Each core computes q/k/v for its heads, causal attention in
S^T layout (keys on partitions, queries on the free dim), and a partial
output projection y_heads @ w_proj[head_rows, :].  The host sums the four
partial projections per batch and adds b_proj.

v2 design (vs the fp32r baseline):
  - All matmul operands are bf16 (full PE rate at any moving size, half
    LDWEIGHTS cost).  PSUM accumulation stays fp32.
  - x is transposed to x^T on the HOST, so the on-chip transpose phase
    (128 PE transposes + 128 PSUM->SBUF copies) disappears.
  - One software-pipelined group loop keeps the PE continuously busy so it
    ramps to / stays at the 2.4 GHz p-state:
        warmup mms | V(0) QK(0) | attn(p0,0) attn(p1,0) V(1) QK(1) proj(0)
                   | attn(p0,1) ... proj(1) | ... | attn(p1,3) proj(3)
    Group g only needs x^T tiles 4g..4g+3, so DMA streams ahead of compute.
  - Attention is flash-style per 512-query chunk: S^T matmul pairs (head A
    rows 0:64 / head B rows 64:128 via tile_position), exp on the Scalar
    engine with the 1/sqrt(hd) scale FUSED into the activation, a 2-deep
    PV lag so exp latency is hidden, and the softmax denominator produced
    by an extra ones-column in V (output row 64 of the PV accumulators).
  - PSUM: 3 rotating 2-bank "st" slots (scores + V/QK/proj) + 2 1-bank
    "y" slots (PV accumulators) = exactly 8 banks.
  - Normalization: reciprocal_approx_fast on the denominator strip, gpsimd
    broadcast across partitions, vector multiply into bf16 y^T.
"""

import os
import sys

import numpy as np
import ml_dtypes

for _p in ("/opt/trn_rl_repo", "/root/.axon_site/_ro/trn_rl_repo"):
    if _p not in sys.path and os.path.isdir(_p):
        sys.path.append(_p)

import concourse.bass as bass  # noqa: E402
import concourse.tile as tile  # noqa: E402
from concourse import bacc, mybir  # noqa: E402
from concourse.bass_utils import run_bass_kernel_spmd  # noqa: E402

P = 128
B, T, C = 2, 2048, 1024
NH, HD = 16, 64
HPC = 4  # heads per core
FPC = HPC * HD  # features per core (256)
QCW = 512  # query-chunk width
NT = T // P  # 16 token tiles
NCB = C // P  # 8 contraction blocks
NQC = T // QCW  # 4 query chunks / groups
F32 = mybir.dt.float32
BF16 = mybir.dt.bfloat16
BFNP = ml_dtypes.bfloat16


def build_nc(debug: bool = False):
    nc = bacc.Bacc("TRN2", target_bir_lowering=False, debug=False)

    xt_d = nc.dram_tensor("xt", [C, T], BF16, kind="ExternalInput")
    wq_d = nc.dram_tensor("wq", [C, FPC], BF16, kind="ExternalInput")
    wk_d = nc.dram_tensor("wk", [C, FPC], BF16, kind="ExternalInput")
    wv_d = nc.dram_tensor("wv", [C, FPC], BF16, kind="ExternalInput")
    bqk_d = nc.dram_tensor("bqk", [P, 4], F32, kind="ExternalInput")
    bv_d = nc.dram_tensor("bv", [1, FPC], BF16, kind="ExternalInput")
    wp_d = nc.dram_tensor("wp", [FPC, C], BF16, kind="ExternalInput")
    triu2_d = nc.dram_tensor("triu2", [P, 2 * P], BF16, kind="ExternalInput")
    ones_d = nc.dram_tensor("ones", [P, QCW], BF16, kind="ExternalInput")
    out_d = nc.dram_tensor("out", [T, C], F32, kind="ExternalOutput")

    from contextlib import ExitStack

    with tile.TileContext(nc) as tc, ExitStack() as ctx:
        consts = ctx.enter_context(tc.tile_pool(name="consts", bufs=1))
        bigs = ctx.enter_context(tc.tile_pool(name="bigs", bufs=1))
        epool = ctx.enter_context(tc.tile_pool(name="epool", bufs=4))
        smalls = ctx.enter_context(tc.tile_pool(name="smalls", bufs=2))
        stage = ctx.enter_context(tc.tile_pool(name="stage", bufs=3))
        pst = ctx.enter_context(tc.tile_pool(name="pst", bufs=3, space="PSUM"))
        py = ctx.enter_context(tc.tile_pool(name="py", bufs=2, space="PSUM"))

        # ---- constants / weights into SBUF ----
        triu2 = consts.tile([P, 2, P], BF16)
        ones = consts.tile([P, QCW], BF16)
        bqk = consts.tile([P, 4], F32)
        bv = consts.tile([1, FPC], BF16)
        nc.sync.dma_start(out=triu2, in_=triu2_d.ap().rearrange("p (a q) -> p a q", a=2))
        nc.sync.dma_start(out=ones, in_=ones_d.ap())
        nc.sync.dma_start(out=bqk, in_=bqk_d.ap())
        nc.sync.dma_start(out=bv, in_=bv_d.ap())

        wq_sb = bigs.tile([P, NCB, FPC], BF16, tag="wq")
        wk_sb = bigs.tile([P, NCB, FPC], BF16, tag="wk")
        wv_sb = bigs.tile([P, NCB, FPC], BF16, tag="wv")
        nc.sync.dma_start(out=wv_sb, in_=wv_d.ap().rearrange("(cb p) f -> p cb f", p=P))

        xt = bigs.tile([P, NCB, T], BF16, tag="xt")
        # x^T arrives in 4 column-group DMAs so group 0 lands early.
        xt_view = xt_d.ap().rearrange("(cb p) t -> p cb t", p=P)
        nc.sync.dma_start(out=xt[:, :, 0:QCW], in_=xt_view[:, :, 0:QCW])
        nc.sync.dma_start(out=wq_sb, in_=wq_d.ap().rearrange("(cb p) f -> p cb f", p=P))
        nc.sync.dma_start(out=wk_sb, in_=wk_d.ap().rearrange("(cb p) f -> p cb f", p=P))
        for g in range(1, NQC):
            cs = slice(g * QCW, (g + 1) * QCW)
            nc.sync.dma_start(out=xt[:, :, cs], in_=xt_view[:, :, cs])
        wp_sb = bigs.tile([P, 2, C], BF16, tag="wp")
        nc.sync.dma_start(out=wp_sb, in_=wp_d.ap().rearrange("(fb p) o -> p fb o", p=P))

        # ---- PE warmup: keep the p-state ramping while DMAs land ----
        warm_act = smalls.tile([1, 16], F32, tag="warm_act", bufs=1)
        nc.scalar.activation(warm_act, ones[0:1, 0:16], mybir.ActivationFunctionType.Exp)
        for i in range(20):
            wps = pst.tile([P, QCW], F32, tag="ps", name=f"warm{i}")
            nc.tensor.matmul(wps, ones[:, 0:P], ones, start=True, stop=True)

        # ---- persistent activations ----
        qt = [bigs.tile([P, T], BF16, tag=f"qt{i}", name=f"qt{i}") for i in range(2)]
        kt = [bigs.tile([P, T], BF16, tag=f"kt{i}", name=f"kt{i}") for i in range(2)]
        yt = [bigs.tile([P, T], BF16, tag=f"yt{i}", name=f"yt{i}") for i in range(2)]
        # V with a ones column per head: PV then also yields the softmax
        # denominator in output row 64 (M=65).
        v_sb = bigs.tile([P, NT, 2, 130], BF16, tag="v")
        for h in (64, 129):
            nc.vector.tensor_copy(
                out=v_sb[:, :, :, h],
                in_=ones[:, 0 : NT * 2].rearrange("p (a b) -> p a b", b=2),
            )

        def v_group(g):
            for t in range(4 * g, 4 * g + 4):
                ps = pst.tile([P, FPC], F32, tag="ps", name="vps")
                for cb in range(NCB):
                    nc.tensor.matmul(
                        ps,
                        xt[:, cb, t * P : (t + 1) * P],
                        wv_sb[:, cb, :],
                        start=(cb == 0),
                        stop=False,
                    )
                nc.tensor.matmul(
                    ps, ones[0:1, 0:P], bv, start=False, stop=True
                )
                nc.vector.tensor_copy(
                    out=v_sb[:, t].rearrange("p a (h w) -> p a h w", w=65)[
                        :, :, :, 0:64
                    ],
                    in_=ps.rearrange("p (a h w) -> p a h w", a=2, w=64),
                )

        def qk_group(g):
            cs = slice(g * QCW, (g + 1) * QCW)
            for wsb, dst, bi in ((wq_sb, qt, 0), (wk_sb, kt, 2)):
                for pair in range(2):
                    fs = slice(pair * P, (pair + 1) * P)
                    ps = pst.tile([P, QCW], F32, tag="ps", name="qkps")
                    for cb in range(NCB):
                        nc.tensor.matmul(
                            ps,
                            wsb[:, cb, fs],
                            xt[:, cb, cs],
                            start=(cb == 0),
                            stop=(cb == NCB - 1),
                        )
                    nc.vector.tensor_scalar_add(
                        dst[pair][:, cs], ps, bqk[:, bi + pair : bi + pair + 1]
                    )

        def attn_chunk(pair, qc):
            cs = slice(qc * QCW, (qc + 1) * QCW)
            cs0 = qc * QCW
            nki = 4 * (qc + 1)
            yA = py.tile([P, QCW], F32, tag="y", name="yA")
            yB = py.tile([P, QCW], F32, tag="y", name="yB")
            pv_pending = []

            def flush_pv():
                ki, lo, e = pv_pending.pop(0)
                st_, sp_ = ki == 0, ki == nki - 1
                nc.tensor.matmul(
                    yA[0:65, lo:],
                    v_sb[:, ki, pair, 0:65],
                    e[:, 0, lo:],
                    start=st_,
                    stop=sp_,
                )
                nc.tensor.matmul(
                    yB[0:65, lo:],
                    v_sb[:, ki, pair, 65:130],
                    e[:, 1, lo:],
                    start=st_,
                    stop=sp_,
                )

            for ki in range(nki):
                m = ki - 4 * qc
                lo = max(m, 0) * P
                ks = slice(ki * P, (ki + 1) * P)
                st = pst.tile([P, 2, QCW], F32, tag="ps", name="st")
                nc.tensor.matmul(
                    st[:, 0, lo:],
                    kt[pair][0:64, ks],
                    qt[pair][0:64, cs0 + lo : cs0 + QCW],
                    start=True,
                    stop=True,
                )
                nc.tensor.matmul(
                    st[:, 1, lo:],
                    kt[pair][64:P, ks],
                    qt[pair][64:P, cs0 + lo : cs0 + QCW],
                    start=True,
                    stop=True,
                    tile_position=(64, 0),
                )
                e = epool.tile([P, 2, QCW], BF16, tag="e", name="e")
                nc.scalar.activation(
                    e[:, :, lo:],
                    st[:, :, lo:],
                    mybir.ActivationFunctionType.Exp,
                    scale=0.125,
                )
                if m >= 0:  # diagonal 128-block: causal triangle mask
                    ds_ = slice(m * P, (m + 1) * P)
                    nc.vector.tensor_mul(e[:, :, ds_], e[:, :, ds_], triu2)
                if debug and pair == 0 and qc == 0 and ki == 0:
                    dbg_e0 = smalls.tile([P, 2, QCW], BF16, tag="dbg_e0", bufs=1)
                    nc.vector.tensor_copy(out=dbg_e0, in_=e)
                    d_e0 = nc.dram_tensor(
                        "dbg_e0", [P, 2 * QCW], BF16, kind="ExternalOutput"
                    )
                    nc.sync.dma_start(
                        out=d_e0.ap().rearrange("p (a q) -> p a q", a=2), in_=dbg_e0
                    )
                pv_pending.append((ki, lo, e))
                if len(pv_pending) > 2:
                    flush_pv()
            while pv_pending:
                flush_pv()

            # normalization chain (hidden under the next group's V/QK work)
            yuA = smalls.tile([65, QCW], F32, tag="yuA")
            yuB = smalls.tile([65, QCW], F32, tag="yuB")
            nc.vector.tensor_copy(out=yuA, in_=yA[0:65, :])
            nc.vector.tensor_copy(out=yuB, in_=yB[0:65, :])
            # 1/s via exp(-ln(s)) on the Scalar engine: Ln and Exp share one
            # activation table, and DVE reciprocal costs 3.3us per strip.
            lnA = smalls.tile([1, QCW], F32, tag="lnA")
            lnB = smalls.tile([1, QCW], F32, tag="lnB")
            recA = smalls.tile([1, QCW], F32, tag="recA")
            recB = smalls.tile([1, QCW], F32, tag="recB")
            nc.scalar.activation(lnA, yuA[64:65, :], mybir.ActivationFunctionType.Ln)
            nc.scalar.activation(lnB, yuB[64:65, :], mybir.ActivationFunctionType.Ln)
            nc.scalar.activation(
                recA, lnA, mybir.ActivationFunctionType.Exp, scale=-1.0
            )
            nc.scalar.activation(
                recB, lnB, mybir.ActivationFunctionType.Exp, scale=-1.0
            )
            recbA = smalls.tile([64, QCW], F32, tag="recbA")
            recbB = smalls.tile([64, QCW], F32, tag="recbB")
            nc.gpsimd.dma_start(
                out=recbA, in_=recA[0:1, None, :].broadcast_to([1, 64, QCW])
            )
            nc.gpsimd.dma_start(
                out=recbB, in_=recB[0:1, None, :].broadcast_to([1, 64, QCW])
            )
            nc.vector.tensor_mul(yt[pair][0:64, cs], yuA[0:64, :], recbA)
            nc.vector.tensor_mul(yt[pair][64:P, cs], yuB[0:64, :], recbB)
            if debug and pair == 0 and qc == 0:
                for nm, src in (
                    ("dbg_yuA", yuA),
                    ("dbg_recA", recA),
                    ("dbg_recbA", recbA),
                ):
                    d = nc.dram_tensor(
                        nm, [src.partition_size(), QCW], F32, kind="ExternalOutput"
                    )
                    dtile = smalls.tile(
                        [src.partition_size(), QCW], F32, tag=nm, bufs=1, name=nm
                    )
                    nc.vector.tensor_copy(out=dtile, in_=src)
                    nc.sync.dma_start(out=d.ap(), in_=dtile)

        def proj_group(g):
            for t in range(4 * g, 4 * g + 4):
                ost = stage.tile([P, C], F32, tag="stage")
                ps = pst.tile([P, 2, QCW], F32, tag="ps", name="pjps")
                for nch in range(2):
                    for fb in range(2):
                        nc.tensor.matmul(
                            ps[:, nch, :],
                            yt[fb][:, t * P : (t + 1) * P],
                            wp_sb[:, fb, nch * QCW : (nch + 1) * QCW],
                            start=(fb == 0),
                            stop=(fb == 1),
                        )
                nc.vector.tensor_copy(
                    out=ost.rearrange("p (a q) -> p a q", a=2), in_=ps
                )
                nc.sync.dma_start(out=out_d.ap()[t * P : (t + 1) * P, :], in_=ost)

        # ---- the pipelined group loop ----
        v_group(0)
        qk_group(0)
        for g in range(NQC):
            attn_chunk(0, g)
            attn_chunk(1, g)
            if g < NQC - 1:
                v_group(g + 1)
                qk_group(g + 1)
            proj_group(g)

        if debug:
            for nm, src in (
                ("dbg_qt0", qt[0]),
                ("dbg_kt0", kt[0]),
                ("dbg_yt0", yt[0]),
                ("dbg_v", v_sb.rearrange("p a b c -> p (a b c)")),
                ("dbg_xt0", xt[:, 0, :]),
            ):
                d = nc.dram_tensor(
                    nm, [P, src.free_size()], src.dtype, kind="ExternalOutput"
                )
                nc.sync.dma_start(out=d.ap(), in_=src)

    nc.compile()
    return nc


_NC_CACHE: dict = {}
LAST_RESULT = None


def kernel(x, w_attn, b_attn, w_proj, b_proj):
    global LAST_RESULT
    x = np.asarray(x, np.float32)
    w_attn = np.asarray(w_attn, np.float32)
    b_attn = np.asarray(b_attn, np.float32)
    w_proj = np.asarray(w_proj, np.float32)
    b_proj = np.asarray(b_proj, np.float32)

    if "nc" not in _NC_CACHE:
        _NC_CACHE["nc"] = build_nc(debug=bool(os.environ.get("BASS_DEBUG")))
    nc = _NC_CACHE["nc"]

    triu = np.triu(np.ones((P, P), np.float32))
    triu2 = np.concatenate([triu, triu], axis=1).astype(BFNP)
    ones = np.ones((P, QCW), np.float32).astype(BFNP)

    in_maps = []
    for core in range(8):
        b, g = core // 4, core % 4
        f0 = g * FPC
        bqk = np.stack(
            [
                b_attn[f0 : f0 + P],
                b_attn[f0 + P : f0 + FPC],
                b_attn[C + f0 : C + f0 + P],
                b_attn[C + f0 + P : C + f0 + FPC],
            ],
            axis=1,
        ).astype(np.float32)
        in_maps.append(
            {
                "xt": np.ascontiguousarray(x[b].T).astype(BFNP),
                "wq": w_attn[:, f0 : f0 + FPC].astype(BFNP),
                "wk": w_attn[:, C + f0 : C + f0 + FPC].astype(BFNP),
                "wv": w_attn[:, 2 * C + f0 : 2 * C + f0 + FPC].astype(BFNP),
                "bqk": np.ascontiguousarray(bqk),
                "bv": b_attn[2 * C + f0 : 2 * C + f0 + FPC][None, :].astype(BFNP),
                "wp": w_proj[f0 : f0 + FPC, :].astype(BFNP),
                "triu2": triu2,
                "ones": ones,
            }
        )

    trace = bool(os.environ.get("BASS_TRACE"))
    res = run_bass_kernel_spmd(
        nc,
        in_maps,
        core_ids=list(range(8)),
        trace=trace,
        tmpdir=os.environ.get("KERNEL_TRACE_DIR") or None,
    )
    LAST_RESULT = res

    y = np.empty((B, T, C), np.float32)
    for b in range(B):
        acc = res.results[4 * b]["out"].astype(np.float32).copy()
        for g in range(1, 4):
            acc += res.results[4 * b + g]["out"]
        y[b] = acc + b_proj[None, :]
    return y


# revision 12
# speedup vs baseline: 1.6754x; 1.0593x over previous
"""Causal self-attention (B=2, T=2048, C=1024, NH=16) on 8 trn2 NeuronCores.

Sharding: core c handles batch b = c//4 and head group g = c%4 (4 heads,
256 features).  Each core computes q/k/v for its heads, causal attention in
S^T layout (keys on partitions, queries on the free dim), and a partial
output projection y_heads @ w_proj[head_rows, :].  The host sums the four
partial projections per batch and adds b_proj.

v2 design (vs the fp32r baseline):
  - All matmul operands are bf16 (full PE rate at any moving size, half
    LDWEIGHTS cost).  PSUM accumulation stays fp32.
  - x is transposed to x^T on the HOST, so the on-chip transpose phase
    (128 PE transposes + 128 PSUM->SBUF copies) disappears.
  - One software-pipelined group loop keeps the PE continuously busy so it
    ramps to / stays at the 2.4 GHz p-state:
        warmup mms | V(0) QK(0) | attn(p0,0) attn(p1,0) V(1) QK(1) proj(0)
                   | attn(p0,1) ... proj(1) | ... | attn(p1,3) proj(3)
    Group g only needs x^T tiles 4g..4g+3, so DMA streams ahead of compute.
  - Attention is flash-style per 512-query chunk: S^T matmul pairs (head A
    rows 0:64 / head B rows 64:128 via tile_position), exp on the Scalar
    engine with the 1/sqrt(hd) scale FUSED into the activation, a 2-deep
    PV lag so exp latency is hidden, and the softmax denominator produced
    by an extra ones-column in V (output row 64 of the PV accumulators).
  - PSUM: 3 rotating 2-bank "st" slots (scores + V/QK/proj) + 2 1-bank
    "y" slots (PV accumulators) = exactly 8 banks.
  - Normalization: reciprocal_approx_fast on the denominator strip, gpsimd
    broadcast across partitions, vector multiply into bf16 y^T.
"""

import os
import sys

import numpy as np
import ml_dtypes

for _p in ("/opt/trn_rl_repo", "/root/.axon_site/_ro/trn_rl_repo"):
    if _p not in sys.path and os.path.isdir(_p):
        sys.path.append(_p)

import concourse.bass as bass  # noqa: E402
import concourse.tile as tile  # noqa: E402
from concourse import bacc, mybir  # noqa: E402
from concourse.bass_utils import run_bass_kernel_spmd  # noqa: E402

P = 128
B, T, C = 2, 2048, 1024
NH, HD = 16, 64
HPC = 4  # heads per core
FPC = HPC * HD  # features per core (256)
QCW = 512  # query-chunk width
NT = T // P  # 16 token tiles
NCB = C // P  # 8 contraction blocks
NQC = T // QCW  # 4 query chunks / groups
F32 = mybir.dt.float32
BF16 = mybir.dt.bfloat16
BFNP = ml_dtypes.bfloat16


def _pin_act_table(nc):
    """Force every activation onto the natural_log_exp_and_others table (it
    serves Exp, Ln, Copy and Identity) so the scalar engine loads ONE table
    instead of flip-flopping between the Exp and Ln tables (1.28us per
    reload).  Table indices are preserved; non-matching tables are just
    emptied so the chooser cannot pick them."""
    import types
    import bass_rust
    from concourse.hw_specs import get_activation_tables
    from concourse import mybir as mb

    def patched(self):
        has_activation = any(
            isinstance(i, mb.InstActivation)
            for b in self.main_func.blocks
            for i in b.instructions
        )
        if not has_activation:
            return
        tables = []
        for name, funcs in get_activation_tables(self.m.arch).items():
            tables.append((name, funcs if name == "natural_log_exp_and_others" else set()))
        bass_rust.insert_act_table_loads(self, tables)

    nc.insert_act_table_loads = types.MethodType(patched, nc)


def build_nc(debug: bool = False):
    nc = bacc.Bacc("TRN2", target_bir_lowering=False, debug=False)
    _pin_act_table(nc)

    xt_d = nc.dram_tensor("xt", [C, T], BF16, kind="ExternalInput")
    wq_d = nc.dram_tensor("wq", [C, FPC], BF16, kind="ExternalInput")
    wk_d = nc.dram_tensor("wk", [C, FPC], BF16, kind="ExternalInput")
    wv_d = nc.dram_tensor("wv", [C, FPC], BF16, kind="ExternalInput")
    bqk_d = nc.dram_tensor("bqk", [P, 4], F32, kind="ExternalInput")
    bv_d = nc.dram_tensor("bv", [1, FPC], BF16, kind="ExternalInput")
    wp_d = nc.dram_tensor("wp", [FPC, C], BF16, kind="ExternalInput")
    triu2_d = nc.dram_tensor("triu2", [P, 2 * P], BF16, kind="ExternalInput")
    ones_d = nc.dram_tensor("ones", [P, QCW], BF16, kind="ExternalInput")
    out_d = nc.dram_tensor("out", [T, C], F32, kind="ExternalOutput")

    from contextlib import ExitStack

    with tile.TileContext(nc) as tc, ExitStack() as ctx:
        consts = ctx.enter_context(tc.tile_pool(name="consts", bufs=1))
        bigs = ctx.enter_context(tc.tile_pool(name="bigs", bufs=1))
        epool = ctx.enter_context(tc.tile_pool(name="epool", bufs=4))
        smalls = ctx.enter_context(tc.tile_pool(name="smalls", bufs=2))
        stage = ctx.enter_context(tc.tile_pool(name="stage", bufs=3))
        pst = ctx.enter_context(tc.tile_pool(name="pst", bufs=3, space="PSUM"))
        py = ctx.enter_context(tc.tile_pool(name="py", bufs=2, space="PSUM"))

        # ---- constants / weights into SBUF ----
        triu2 = consts.tile([P, 2, P], BF16)
        ones = consts.tile([P, QCW], BF16)
        bqk = consts.tile([P, 4], F32)
        bv = consts.tile([1, FPC], BF16)
        nc.sync.dma_start(out=triu2, in_=triu2_d.ap().rearrange("p (a q) -> p a q", a=2))
        nc.sync.dma_start(out=ones, in_=ones_d.ap())
        nc.sync.dma_start(out=bqk, in_=bqk_d.ap())
        nc.sync.dma_start(out=bv, in_=bv_d.ap())

        wq_sb = bigs.tile([P, NCB, FPC], BF16, tag="wq")
        wk_sb = bigs.tile([P, NCB, FPC], BF16, tag="wk")
        wv_sb = bigs.tile([P, NCB, FPC], BF16, tag="wv")
        nc.sync.dma_start(out=wv_sb, in_=wv_d.ap().rearrange("(cb p) f -> p cb f", p=P))

        xt = bigs.tile([P, NCB, T], BF16, tag="xt")
        # x^T arrives as 16 parallel DMAs (4 cb-pairs x 4 column groups):
        # 1KB contiguous DRAM lines, and ~16 DMA engines run concurrently.
        xt_view = xt_d.ap().rearrange("(cb p) t -> p cb t", p=P)
        for cb2 in range(NCB // 2):
            cbs = slice(2 * cb2, 2 * cb2 + 2)
            nc.sync.dma_start(
                out=xt[:, cbs, 0:QCW], in_=xt_view[:, cbs, 0:QCW]
            )
        nc.sync.dma_start(out=wq_sb, in_=wq_d.ap().rearrange("(cb p) f -> p cb f", p=P))
        nc.sync.dma_start(out=wk_sb, in_=wk_d.ap().rearrange("(cb p) f -> p cb f", p=P))
        for g in range(1, NQC):
            cs = slice(g * QCW, (g + 1) * QCW)
            for cb2 in range(NCB // 2):
                cbs = slice(2 * cb2, 2 * cb2 + 2)
                nc.sync.dma_start(out=xt[:, cbs, cs], in_=xt_view[:, cbs, cs])
        wp_sb = bigs.tile([P, 2, C], BF16, tag="wp")
        nc.sync.dma_start(out=wp_sb, in_=wp_d.ap().rearrange("(fb p) o -> p fb o", p=P))

        # ---- PE warmup: keep the p-state ramping while DMAs land ----
        warm_act = smalls.tile([1, 16], F32, tag="warm_act", bufs=1)
        nc.scalar.activation(warm_act, ones[0:1, 0:16], mybir.ActivationFunctionType.Exp)
        for i in range(20):
            wps = pst.tile([P, QCW], F32, tag="ps", name=f"warm{i}")
            nc.tensor.matmul(wps, ones[:, 0:P], ones, start=True, stop=True)

        # ---- persistent activations ----
        qt = [bigs.tile([P, T], BF16, tag=f"qt{i}", name=f"qt{i}") for i in range(2)]
        kt = [bigs.tile([P, T], BF16, tag=f"kt{i}", name=f"kt{i}") for i in range(2)]
        yt = [bigs.tile([P, T], BF16, tag=f"yt{i}", name=f"yt{i}") for i in range(2)]
        # V with a ones column per head: PV then also yields the softmax
        # denominator in output row 64 (M=65).
        v_sb = bigs.tile([P, NT, 2, 130], BF16, tag="v")
        for h in (64, 129):
            nc.vector.tensor_copy(
                out=v_sb[:, :, :, h],
                in_=ones[:, 0 : NT * 2].rearrange("p (a b) -> p a b", b=2),
            )

        def v_group(g):
            for t in range(4 * g, 4 * g + 4):
                ps = pst.tile([P, FPC], F32, tag="ps", name="vps")
                for cb in range(NCB):
                    nc.tensor.matmul(
                        ps,
                        xt[:, cb, t * P : (t + 1) * P],
                        wv_sb[:, cb, :],
                        start=(cb == 0),
                        stop=False,
                    )
                nc.tensor.matmul(
                    ps, ones[0:1, 0:P], bv, start=False, stop=True
                )
                nc.vector.tensor_copy(
                    out=v_sb[:, t].rearrange("p a (h w) -> p a h w", w=65)[
                        :, :, :, 0:64
                    ],
                    in_=ps.rearrange("p (a h w) -> p a h w", a=2, w=64),
                )

        def qk_group(g):
            cs = slice(g * QCW, (g + 1) * QCW)
            for wsb, dst, bi in ((wq_sb, qt, 0), (wk_sb, kt, 2)):
                for pair in range(2):
                    fs = slice(pair * P, (pair + 1) * P)
                    ps = pst.tile([P, QCW], F32, tag="ps", name="qkps")
                    for cb in range(NCB):
                        nc.tensor.matmul(
                            ps,
                            wsb[:, cb, fs],
                            xt[:, cb, cs],
                            start=(cb == 0),
                            stop=(cb == NCB - 1),
                        )
                    nc.vector.tensor_scalar_add(
                        dst[pair][:, cs], ps, bqk[:, bi + pair : bi + pair + 1]
                    )

        def attn_chunk(pair, qc):
            cs = slice(qc * QCW, (qc + 1) * QCW)
            cs0 = qc * QCW
            nki = 4 * (qc + 1)
            yA = py.tile([P, QCW], F32, tag="y", name="yA")
            yB = py.tile([P, QCW], F32, tag="y", name="yB")
            pv_pending = []

            def flush_pv():
                ki, lo, e = pv_pending.pop(0)
                st_, sp_ = ki == 0, ki == nki - 1
                nc.tensor.matmul(
                    yA[0:65, lo:],
                    v_sb[:, ki, pair, 0:65],
                    e[:, 0, lo:],
                    start=st_,
                    stop=sp_,
                )
                nc.tensor.matmul(
                    yB[0:65, lo:],
                    v_sb[:, ki, pair, 65:130],
                    e[:, 1, lo:],
                    start=st_,
                    stop=sp_,
                )

            for ki in range(nki):
                m = ki - 4 * qc
                lo = max(m, 0) * P
                ks = slice(ki * P, (ki + 1) * P)
                st = pst.tile([P, 2, QCW], F32, tag="ps", name="st")
                nc.tensor.matmul(
                    st[:, 0, lo:],
                    kt[pair][0:64, ks],
                    qt[pair][0:64, cs0 + lo : cs0 + QCW],
                    start=True,
                    stop=True,
                )
                nc.tensor.matmul(
                    st[:, 1, lo:],
                    kt[pair][64:P, ks],
                    qt[pair][64:P, cs0 + lo : cs0 + QCW],
                    start=True,
                    stop=True,
                    tile_position=(64, 0),
                )
                e = epool.tile([P, 2, QCW], BF16, tag="e", name="e")
                nc.scalar.activation(
                    e[:, :, lo:],
                    st[:, :, lo:],
                    mybir.ActivationFunctionType.Exp,
                    scale=0.125,
                )
                if m >= 0:  # diagonal 128-block: causal triangle mask
                    ds_ = slice(m * P, (m + 1) * P)
                    nc.vector.tensor_mul(e[:, :, ds_], e[:, :, ds_], triu2)
                if debug and pair == 0 and qc == 0 and ki == 0:
                    dbg_e0 = smalls.tile([P, 2, QCW], BF16, tag="dbg_e0", bufs=1)
                    nc.vector.tensor_copy(out=dbg_e0, in_=e)
                    d_e0 = nc.dram_tensor(
                        "dbg_e0", [P, 2 * QCW], BF16, kind="ExternalOutput"
                    )
                    nc.sync.dma_start(
                        out=d_e0.ap().rearrange("p (a q) -> p a q", a=2), in_=dbg_e0
                    )
                pv_pending.append((ki, lo, e))
                if len(pv_pending) > 2:
                    flush_pv()
            while pv_pending:
                flush_pv()

            # normalization chain (hidden under the next group's V/QK work)
            yuA = smalls.tile([65, QCW], F32, tag="yuA")
            yuB = smalls.tile([65, QCW], F32, tag="yuB")
            nc.vector.tensor_copy(out=yuA, in_=yA[0:65, :])
            nc.vector.tensor_copy(out=yuB, in_=yB[0:65, :])
            # 1/s via exp(-ln(s)) on the Scalar engine: Ln and Exp share one
            # activation table, and DVE reciprocal costs 3.3us per strip.
            lnA = smalls.tile([1, QCW], F32, tag="lnA")
            lnB = smalls.tile([1, QCW], F32, tag="lnB")
            recA = smalls.tile([1, QCW], F32, tag="recA")
            recB = smalls.tile([1, QCW], F32, tag="recB")
            nc.scalar.activation(lnA, yuA[64:65, :], mybir.ActivationFunctionType.Ln)
            nc.scalar.activation(lnB, yuB[64:65, :], mybir.ActivationFunctionType.Ln)
            nc.scalar.activation(
                recA, lnA, mybir.ActivationFunctionType.Exp, scale=-1.0
            )
            nc.scalar.activation(
                recB, lnB, mybir.ActivationFunctionType.Exp, scale=-1.0
            )
            recbA = smalls.tile([64, QCW], F32, tag="recbA")
            recbB = smalls.tile([64, QCW], F32, tag="recbB")
            nc.gpsimd.dma_start(
                out=recbA, in_=recA[0:1, None, :].broadcast_to([1, 64, QCW])
            )
            nc.gpsimd.dma_start(
                out=recbB, in_=recB[0:1, None, :].broadcast_to([1, 64, QCW])
            )
            nc.vector.tensor_mul(yt[pair][0:64, cs], yuA[0:64, :], recbA)
            nc.vector.tensor_mul(yt[pair][64:P, cs], yuB[0:64, :], recbB)
            if debug and pair == 0 and qc == 0:
                for nm, src in (
                    ("dbg_yuA", yuA),
                    ("dbg_recA", recA),
                    ("dbg_recbA", recbA),
                ):
                    d = nc.dram_tensor(
                        nm, [src.partition_size(), QCW], F32, kind="ExternalOutput"
                    )
                    dtile = smalls.tile(
                        [src.partition_size(), QCW], F32, tag=nm, bufs=1, name=nm
                    )
                    nc.vector.tensor_copy(out=dtile, in_=src)
                    nc.sync.dma_start(out=d.ap(), in_=dtile)

        def proj_group(g):
            for t in range(4 * g, 4 * g + 4):
                ost = stage.tile([P, C], F32, tag="stage")
                ps = pst.tile([P, 2, QCW], F32, tag="ps", name="pjps")
                for nch in range(2):
                    for fb in range(2):
                        nc.tensor.matmul(
                            ps[:, nch, :],
                            yt[fb][:, t * P : (t + 1) * P],
                            wp_sb[:, fb, nch * QCW : (nch + 1) * QCW],
                            start=(fb == 0),
                            stop=(fb == 1),
                        )
                nc.vector.tensor_copy(
                    out=ost.rearrange("p (a q) -> p a q", a=2), in_=ps
                )
                # two half-height DMAs -> two DMA engines per tile
                nc.sync.dma_start(
                    out=out_d.ap()[t * P : t * P + 64, :], in_=ost[0:64, :]
                )
                nc.sync.dma_start(
                    out=out_d.ap()[t * P + 64 : (t + 1) * P, :], in_=ost[64:P, :]
                )

        # ---- the pipelined group loop ----
        v_group(0)
        qk_group(0)
        for g in range(NQC):
            attn_chunk(0, g)
            attn_chunk(1, g)
            if g < NQC - 1:
                v_group(g + 1)
                qk_group(g + 1)
            proj_group(g)

        if debug:
            for nm, src in (
                ("dbg_qt0", qt[0]),
                ("dbg_kt0", kt[0]),
                ("dbg_yt0", yt[0]),
                ("dbg_v", v_sb.rearrange("p a b c -> p (a b c)")),
                ("dbg_xt0", xt[:, 0, :]),
            ):
                d = nc.dram_tensor(
                    nm, [P, src.free_size()], src.dtype, kind="ExternalOutput"
                )
                nc.sync.dma_start(out=d.ap(), in_=src)

    nc.compile()
    return nc


_NC_CACHE: dict = {}
LAST_RESULT = None


def kernel(x, w_attn, b_attn, w_proj, b_proj):
    global LAST_RESULT
    x = np.asarray(x, np.float32)
    w_attn = np.asarray(w_attn, np.float32)
    b_attn = np.asarray(b_attn, np.float32)
    w_proj = np.asarray(w_proj, np.float32)
    b_proj = np.asarray(b_proj, np.float32)

    if "nc" not in _NC_CACHE:
        _NC_CACHE["nc"] = build_nc(debug=bool(os.environ.get("BASS_DEBUG")))
    nc = _NC_CACHE["nc"]

    triu = np.triu(np.ones((P, P), np.float32))
    triu2 = np.concatenate([triu, triu], axis=1).astype(BFNP)
    ones = np.ones((P, QCW), np.float32).astype(BFNP)

    in_maps = []
    for core in range(8):
        b, g = core // 4, core % 4
        f0 = g * FPC
        bqk = np.stack(
            [
                b_attn[f0 : f0 + P],
                b_attn[f0 + P : f0 + FPC],
                b_attn[C + f0 : C + f0 + P],
                b_attn[C + f0 + P : C + f0 + FPC],
            ],
            axis=1,
        ).astype(np.float32)
        in_maps.append(
            {
                "xt": np.ascontiguousarray(x[b].T).astype(BFNP),
                "wq": w_attn[:, f0 : f0 + FPC].astype(BFNP),
                "wk": w_attn[:, C + f0 : C + f0 + FPC].astype(BFNP),
                "wv": w_attn[:, 2 * C + f0 : 2 * C + f0 + FPC].astype(BFNP),
                "bqk": np.ascontiguousarray(bqk),
                "bv": b_attn[2 * C + f0 : 2 * C + f0 + FPC][None, :].astype(BFNP),
                "wp": w_proj[f0 : f0 + FPC, :].astype(BFNP),
                "triu2": triu2,
                "ones": ones,
            }
        )

    trace = bool(os.environ.get("BASS_TRACE"))
    res = run_bass_kernel_spmd(
        nc,
        in_maps,
        core_ids=list(range(8)),
        trace=trace,
        tmpdir=os.environ.get("KERNEL_TRACE_DIR") or None,
    )
    LAST_RESULT = res

    y = np.empty((B, T, C), np.float32)
    for b in range(B):
        acc = res.results[4 * b]["out"].astype(np.float32).copy()
        for g in range(1, 4):
            acc += res.results[4 * b + g]["out"]
        y[b] = acc + b_proj[None, :]
    return y
